# revision 35
# baseline (speedup 1.0000x reference)
"""Trainium2 Bass kernel for nn_Aggregate (gnn_message_passing).

Sharding: 8 cores = 2 directions x 4 batch-groups. Cores 0-3 compute
refined_async (source = sync_fea, adj = sync_adj, weights a_*) for 8
batches each; cores 4-7 compute refined_sync. The feature passthrough
(output channels 512:1024) and the no-neighbor fallback select are pure
input data movement, done host-side during unsharding.

Device algorithm per core (8 batches, one direction):
  Activations stay feature-major ([feat, node]); weights host-transposed
  to input-major (matmul lhsT). Batch parity j on partition halves
  (rows j*64+node).

    kT = WkT^T xT + bk ; qT = WqT^T xT + bq  (q pre-scaled 1/sqrt(dh))
    v  = xT^T WvT                            (node-major, bias folded out)
    per (head, batch):  Pq = exp(qT_h^T kT_h)   [q,k]
                        Pk = exp(kT_h^T qT_h)   [k,q]
                        den = Pk^T Af           [q,t]
                        w   = Af / den          [q,t]  (single divide)
                        ST  = Pq^T w            [k,t]
                        SmT = ST * (Af r^2)     [k,t]  (r = 1/cnt, host)
                        G_h = v_h^T SmT         [d,t]
    m2  = (WmWo)^T-contraction @ G  +  [c0|bm] rank-2 @ [r|1]
  The adj-derived normalizers cnt, r=1/max(cnt,1), r^2 are computed on
  host (adj is an input): r^2 rides in as Af*r^2 for the Sm mask-mult,
  and the affine tail c0*r + bm is a K=2 matmul accumulated into m2
  psum. c0 = Wm @ (Wo @ bv + bo): the v-bias folds out exactly because
  sum_k SmT_h[k,t] = cnt[t].

Performance structure (cost-model driven):
  - 3 parallel DMA queues (SP HWDGE / Pool SWDGE / Act HWDGE) so input
    loads don't serialize on the single HWDGE slot.
  - PE warmup matmuls during the load phase keep the Tensor engine
    continuously busy so its p-state ramp (3us to full clock) completes
    before real work; every real matmul then runs at max rate.
  - Scores psum is tiled per (orientation, feature-half) so exp and the
    den/S chain unblock per half; v matmuls interleave between den and
    S as PE filler while the divide chain runs on DVE; final copies are
    split across DVE and Act.

Built on bacc.Bacc: its compile() legalizes sync waits (TRN2 allows one
wait per instruction) via ldweights-wait motion + event semaphores.
"""

import numpy as np

FEA, H, B, N = 256, 8, 32, 48
DH = FEA // H
NB = 8            # batches per core
NPAIR = NB // 2
NCORES = 8
NT = NB * N       # 384

N_WARM = 66       # PE warmup matmuls (ap 64) covering the DMA load phase

_cached = None


def _build_program():
    import concourse.tile as tile
    from concourse.tile import add_dep_helper
    from concourse import bacc, mybir
    from contextlib import ExitStack

    f32 = mybir.dt.float32
    bf = mybir.dt.bfloat16
    AF = mybir.ActivationFunctionType
    OP = mybir.AluOpType

    nc = bacc.Bacc("TRN2", target_bir_lowering=False, debug=False)

    # ---- DRAM I/O ----
    # blobA1: xT both kc-chunks (768) | wk ot0 chunks (256)
    blobA1_d = nc.dram_tensor("blobA1", [128, 1024], bf, kind="ExternalInput")
    blobA2_d = nc.dram_tensor("blobA2", [128, 256], bf, kind="ExternalInput")
    # rcb: [r|1] rows (384 cols) then [c0|bm] rows (256 cols)
    rcb_d = nc.dram_tensor("rcb", [2, 640], bf, kind="ExternalInput")
    consts_d = nc.dram_tensor("consts", [128, 4], f32, kind="ExternalInput")
    # wq, ot-major chunks
    wq_d = nc.dram_tensor("wqT", [128, 512], bf, kind="ExternalInput")
    # adjst (192) | adjst * r^2 (192)
    adj_d = nc.dram_tensor("adj", [128, 384], bf, kind="ExternalInput")
    # blobB: wvT (512) | womT (512)
    blobB_d = nc.dram_tensor("blobB", [128, 1024], bf, kind="ExternalInput")
    out_d = nc.dram_tensor("outT", [128, 2 * NT], bf, kind="ExternalOutput")

    def p_off(pr, h):
        """Pq/Pk column of (pr, head) block: (ot, g//2, g%2, pr, t)."""
        ot, g = h // 4, h % 4
        return ot * 768 + (g // 2) * 384 + (g % 2) * 192 + pr * N

    with ExitStack() as ctx:
        tc = ctx.enter_context(tile.TileContext(nc))
        sb = ctx.enter_context(tc.tile_pool(name="sb", bufs=1))
        wm = ctx.enter_context(tc.tile_pool(name="wm", bufs=1, space="PSUM"))
        ps1 = ctx.enter_context(tc.tile_pool(name="ps1", bufs=3, space="PSUM"))
        ps2 = ctx.enter_context(tc.tile_pool(name="ps2", bufs=2, space="PSUM"))

        # ---- loads on 3 parallel queues ----
        blobA1 = sb.tile([128, 1024], bf, tag="blobA1")
        nc.sync.dma_start(out=blobA1[:, :], in_=blobA1_d.ap()[:, :])
        blobA2 = sb.tile([128, 256], bf, tag="blobA2")
        nc.sync.dma_start(out=blobA2[:, :], in_=blobA2_d.ap()[:, :])
        consts = sb.tile([128, 4], f32, tag="consts")
        nc.sync.dma_start(out=consts[:, :], in_=consts_d.ap()[:, :])
        rcb = sb.tile([2, 640], bf, tag="rcb")
        nc.sync.dma_start(out=rcb[:, :], in_=rcb_d.ap()[:, :])
        wqT = sb.tile([128, 512], bf, tag="wqT")
        nc.scalar.dma_start(out=wqT[:, :], in_=wq_d.ap()[:, :])
        adj = sb.tile([128, 384], bf, tag="adj")
        nc.gpsimd.dma_start(out=adj[:, :], in_=adj_d.ap()[:, :])
        blobB = sb.tile([128, 1024], bf, tag="blobB")
        nc.scalar.dma_start(out=blobB[:, :], in_=blobB_d.ap()[:, :])

        xT = [blobA1[:, kc * 384:(kc + 1) * 384] for kc in range(2)]

        def wk_sl(ot, kc):
            if ot == 0:
                return blobA1[:, 768 + kc * 128:768 + (kc + 1) * 128]
            return blobA2[:, kc * 128:(kc + 1) * 128]

        def wq_sl(ot, kc):
            return wqT[:, ot * 256 + kc * 128:ot * 256 + (kc + 1) * 128]

        adjst = adj[:, 0:192]
        adjst2 = adj[:, 192:384]
        wvT = blobB[:, 0:512]
        womT = blobB[:, 512:1024]
        bk, bq = consts[:, 0:2], consts[:, 2:4]
        rcx = rcb[:, 0:NT]            # rows: [r | ones]
        rcw = rcb[:, 384:640]         # rows: [c0 | bm]

        ones = sb.tile([N, 128], bf, tag="ones")
        nc.vector.memset(ones[:, :], 1.0)

        # ---- PE warmup: keep Tensor busy through the load phase so the
        # p-state ramp finishes before real matmuls. Results unread. ----
        cp = wm.tile([128, 512], f32, tag="cp")
        for i in range(N_WARM):
            nc.tensor.matmul(cp[0:64, 0:64], ones[:, 0:64], ones[:, 0:64],
                             start=True, stop=True)

        # ---- k/q projections (feature-major), psum tile per ot-half;
        # ot0 pair first so their bias TTs and scores unblock earliest.
        # All staged SBUF tensors are split per half/pair: Tile tracks
        # dependencies at tile granularity, so fine tiles keep readers of
        # one half from serializing behind writers of the other. ----
        kqps = {}
        for ot in range(2):
            p = ps2.tile([128, 2, 512], f32, tag="ps2t")
            for bank, (nm, w_sl) in enumerate((("k", wk_sl), ("q", wq_sl))):
                kqps[nm, ot] = p[:, bank, 0:NT]
                for kc in range(2):
                    nc.tensor.matmul(
                        p[:, bank, 0:NT], w_sl(ot, kc), xT[kc][:, :],
                        start=(kc == 0), stop=(kc == 1),
                    )

        # bias adds: ot0 pair on DVE, ot1 pair on Act (parallel chains)
        kqT = {}
        for ot in range(2):
            for nm, bias in (("k", bk), ("q", bq)):
                t = sb.tile([128, NT], bf, tag=f"{nm}T{ot}")
                kqT[nm, ot] = t
                if ot == 0:
                    nc.vector.tensor_tensor(
                        out=t[:, :], in0=kqps[nm, ot],
                        in1=bias[:, 0:1].to_broadcast((128, NT)),
                        op=OP.add,
                    )
                else:
                    nc.scalar.activation(
                        out=t[:, :], in_=kqps[nm, ot], func=AF.Identity,
                        bias=bias[:, 1:2],
                    )

        def head_slice(nm, h, b):
            """[32, 48] lhsT/rhs slice of the per-ot feature-major tile."""
            t = kqT[nm, h // 4]
            return t[(h % 4) * 32:(h % 4) * 32 + 32, b * N:(b + 1) * N]

        def serial_rowgroups(groups):
            """Same-bank psum row-group serialization (in-order, no-cost)."""
            for gi in range(1, len(groups)):
                for i1 in groups[gi]:
                    for i0 in groups[gi - 1]:
                        add_dep_helper(i1.ins, i0.ins, sync=True,
                                       reason="same-bank row-group serialization")

        # ---- scores + exp, per (orientation, feature-half ot) ----
        # psum tile per (orient, ot): bank b holds heads g=2b,2b+1 (PE
        # row-tiles g*32), cols (g%2)*192 + pr*48. exp -> P[orient][ot].
        P = {o: [None, None] for o in ("k", "q")}

        def po(pr, h):
            g = h % 4
            return (g // 2) * 384 + (g % 2) * 192 + pr * N

        def scores_phase(orient, ot):
            lhs, rhs = ("k", "q") if orient == "k" else ("q", "k")
            p = ps2.tile([128, 2, 512], f32, tag="ps2t")
            groups = []
            for g in range(4):
                h = ot * 4 + g
                grp = []
                for j in range(2):
                    for pr in range(NPAIR):
                        b = pr * 2 + j
                        grp.append(nc.tensor.matmul(
                            p[j * 64:j * 64 + N, g // 2,
                              (g % 2) * 192 + pr * N:(g % 2) * 192 + (pr + 1) * N],
                            head_slice(lhs, h, b),
                            head_slice(rhs, h, b),
                            start=True, stop=True,
                            tile_position=(g * 32, j * 64),
                        ))
                groups.append(grp)
            # A psum bank may not take concurrent writes from different
            # PE row-tiles: bank g//2 receives row-tiles g*32 for both
            # g's of its pair, so serialize g=2b+1 after g=2b per bank.
            serial_rowgroups([groups[0], groups[1]])
            serial_rowgroups([groups[2], groups[3]])
            dst = sb.tile([128, 768], bf, tag=f"P{orient}{ot}")
            P[orient][ot] = dst
            nc.scalar.activation(
                out=dst[:, :].rearrange("p (b f) -> p b f", f=384),
                in_=p[:, :, 0:384], func=AF.Exp,
            )

        # ---- den -> w (= Af/den, one divide per pair) ----
        wT = {}

        def den_phase(pr):
            dp = ps1.tile([128, 512], f32, tag="ps1t")
            groups = [[], []]
            for j in range(2):
                for h in range(H):
                    groups[j].append(nc.tensor.matmul(
                        dp[j * 64:j * 64 + N, h * N:(h + 1) * N],
                        P["k"][h // 4][j * 64:j * 64 + N,
                                       po(pr, h):po(pr, h) + N],
                        adjst[j * 64:j * 64 + N, pr * N:(pr + 1) * N],
                        start=True, stop=True,
                    ))
            serial_rowgroups(groups)
            wt = sb.tile([128, 384], bf, tag=f"wT{pr}")
            wT[pr] = wt
            eng = nc.vector if pr % 2 == 0 else nc.gpsimd
            with nc.allow_low_precision(reason="bf16 attn weights; accum fp32"):
                eng.tensor_tensor(
                    out=wt[:, :].rearrange("p (h t) -> p h t", t=N),
                    in0=adjst[:, pr * N:(pr + 1) * N][:, None, :]
                        .to_broadcast((128, H, N)),
                    in1=dp[:, 0:384].rearrange("p (h t) -> p h t", t=N),
                    op=OP.divide,
                )

        # ---- v (node-major: rows j*64+n, half = pr//2), reusing the
        # warmup psum bank half-by-half; copies split Pool / Act ----
        v = [sb.tile([128, 2, 256], bf, tag=f"v{half}", name=f"v{half}")
             for half in range(2)]

        def v_mms(half):
            for bb in range(4 * half, 4 * half + 4):
                pr, j = bb // 2, bb % 2
                for kc in range(2):
                    nc.tensor.matmul(
                        cp[j * 64:j * 64 + N,
                           (pr % 2) * 256:(pr % 2 + 1) * 256],
                        xT[kc][:, bb * N:(bb + 1) * N],
                        wvT[:, kc * 256:(kc + 1) * 256],
                        start=(kc == 0), stop=(kc == 1),
                    )
            if half == 0:
                nc.gpsimd.tensor_copy(
                    out=v[half][:, :, :],
                    in_=cp[:, :].rearrange("p (c o) -> p c o", o=256),
                )
            else:
                nc.scalar.activation(
                    out=v[half][:, :, :],
                    in_=cp[:, :].rearrange("p (c o) -> p c o", o=256),
                    func=AF.Copy,
                )

        # ---- S -> Sm per (pair, head-half): the h0-3 half only needs
        # Pq-ot0, so G bank 0 decouples from the last exp ----
        SmT = {}

        def s_phase(pr):
            sp = ps1.tile([128, 512], f32, tag="ps1t")
            groups = []
            for hf in range(2):
                for j in range(2):
                    grp = []
                    for h in range(hf * 4, hf * 4 + 4):
                        grp.append(nc.tensor.matmul(
                            sp[j * 64:j * 64 + N, h * N:(h + 1) * N],
                            P["q"][hf][j * 64:j * 64 + N,
                                       po(pr, h):po(pr, h) + N],
                            wT[pr][j * 64:j * 64 + N, h * N:h * N + N],
                            start=True, stop=True,
                        ))
                    groups.append(grp)
                smt = sb.tile([128, 4, N], bf, tag=f"SmT{pr}{hf}")
                SmT[pr, hf] = smt
                # Sm = S * (Af r^2): r^2 pooling scale folded into the mask
                eng = nc.vector if hf == 0 else nc.gpsimd
                eng.tensor_tensor(
                    out=smt[:, :, :],
                    in0=sp[:, hf * 192:(hf + 1) * 192]
                        .rearrange("p (h t) -> p h t", t=N),
                    in1=adjst2[:, pr * N:(pr + 1) * N][:, None, :]
                        .to_broadcast((128, 4, N)),
                    op=OP.mult,
                )
            serial_rowgroups(groups)

        # scores-q-ot1 sits between v halves: its psum slot frees via
        # exp-k-ot1 and its Pq half is only consumed by the S h4-7 halves.
        scores_phase("k", 0)
        scores_phase("k", 1)
        scores_phase("q", 0)
        for pr in range(NPAIR):
            den_phase(pr)
        v_mms(0)
        scores_phase("q", 1)
        s_phase(0)
        s_phase(1)
        v_mms(1)
        s_phase(2)
        s_phase(3)

        # ---- G: pooled, feature-major; bank b = h//4 (= kc of Wm@Wo) in
        # its own psum tile so the bank-0 copy starts as soon as its own
        # matmuls finish; cols (pr, j, t) = output order ----
        Gs = [sb.tile([128, NT], bf, tag=f"Gs{b}", name=f"Gs{b}")
              for b in range(2)]
        for bank in range(2):
            gp = ps1.tile([128, 512], f32, tag="ps1t")
            groups = [[], []]
            for j in range(2):
                for pr in range(NPAIR):
                    for hh in range(4):
                        h = bank * 4 + hh
                        groups[j].append(nc.tensor.matmul(
                            gp[hh * 32:hh * 32 + 32,
                               pr * 96 + j * 48:pr * 96 + j * 48 + N],
                            v[pr // 2][j * 64:j * 64 + N, pr % 2,
                                       h * 32:(h + 1) * 32],
                            SmT[pr, bank][j * 64:j * 64 + N, hh, :],
                            start=True, stop=True,
                            tile_position=(j * 64, hh * 32),
                        ))
            serial_rowgroups(groups)
            nc.vector.tensor_copy(out=Gs[bank][:, :], in_=gp[:, 0:NT])

        # ---- m2 = (WmWo)^T-contraction @ Gs + rank-2 affine (c0*r + bm);
        # per-ot psum tiles so ot1's matmuls don't wait on ot0's copy ----
        m2 = [ps1.tile([128, 512], f32, tag="ps1t", name=f"m2_{i}")
              for i in range(2)]
        for kc in range(2):
            for ot in range(2):
                nc.tensor.matmul(
                    m2[ot][:, 0:NT],
                    womT[:, kc * 256 + ot * 128:kc * 256 + (ot + 1) * 128],
                    Gs[kc][:, :],
                    start=(kc == 0), stop=False,
                )
        for ot in range(2):
            nc.tensor.matmul(
                m2[ot][:, 0:NT],
                rcw[:, ot * 128:(ot + 1) * 128],
                rcx[:, :],
                start=False, stop=True,
            )
        osb0 = sb.tile([128, NT], bf, tag="osb0")
        nc.vector.tensor_copy(out=osb0[:, :], in_=m2[0][:, 0:NT])
        nc.sync.dma_start(out=out_d.ap()[:, 0:NT], in_=osb0[:, :])
        osb1 = sb.tile([128, NT], bf, tag="osb1")
        nc.scalar.activation(out=osb1[:, :], in_=m2[1][:, 0:NT], func=AF.Copy)
        nc.scalar.dma_start(out=out_d.ap()[:, NT:2 * NT], in_=osb1[:, :])

    nc.compile()
    return nc


def _get_program():
    global _cached
    if _cached is None:
        _cached = _build_program()
    return _cached


def _prep_core_inputs(x_src, adj, Wq, bq, Wk, bk, Wv, bv, Wo, bo, Wm, bm):
    """Host-side shard prep for one core: 8 batches of one direction.
    Matmul-side tensors are cast to bfloat16 (PSUM accumulation stays fp32;
    the reference's own fp32 noise dominates the resulting error)."""
    import ml_dtypes
    f32 = np.float32
    bf = ml_dtypes.bfloat16
    xT = np.ascontiguousarray(
        np.transpose(x_src, (2, 0, 1)).reshape(FEA, NT)).astype(bf)
    Af = (adj > 0).astype(f32)                       # [NB, 48(k), 48(t)]
    s = 1.0 / np.sqrt(np.float32(DH))

    def ot_chunks(w):   # [256(in), 256(out)] W.T -> [128, 512] (ot, kc)
        wt = np.ascontiguousarray(w)
        return np.concatenate([wt[kc * 128:(kc + 1) * 128, ot * 128:(ot + 1) * 128]
                               for ot in range(2) for kc in range(2)], axis=1)

    def kc_chunks(w):   # [256, 256] W.T -> [128, 512] (kc major, full out)
        wt = np.ascontiguousarray(w)
        return np.concatenate([wt[0:128, :], wt[128:256, :]], axis=1)

    wkT = ot_chunks(Wk.T).astype(bf)
    blobA1 = np.concatenate([xT[0:128, :], xT[128:256, :], wkT[:, 0:256]], axis=1)
    blobA2 = wkT[:, 256:512]
    wqT = ot_chunks(Wq.T * s).astype(bf)
    blobB = np.concatenate(
        [kc_chunks(Wv.T).astype(bf), kc_chunks((Wm @ Wo).T).astype(bf)], axis=1)

    cnt = Af.sum(axis=1)                             # [NB, 48(t)]
    r = (1.0 / np.maximum(cnt, 1.0)).astype(f32)     # [NB, 48]
    r2 = r * r

    adj_blob = np.zeros((128, 384), f32)
    for p in range(NPAIR):
        adj_blob[0:N, p * N:(p + 1) * N] = Af[2 * p]
        adj_blob[64:64 + N, p * N:(p + 1) * N] = Af[2 * p + 1]
        adj_blob[0:N, 192 + p * N:192 + (p + 1) * N] = Af[2 * p] * r2[2 * p]
        adj_blob[64:64 + N, 192 + p * N:192 + (p + 1) * N] = \
            Af[2 * p + 1] * r2[2 * p + 1]

    c0 = (Wm @ (Wo @ bv + bo)).astype(f32)
    consts = np.zeros((128, 4), f32)
    consts[:, 0:2] = bk.reshape(2, 128).T
    consts[:, 2:4] = (bq * s).reshape(2, 128).T

    rcb = np.zeros((2, 640), f32)
    rcb[0, 0:NT] = r.reshape(NT)
    rcb[1, 0:NT] = 1.0
    rcb[0, 384:640] = c0
    rcb[1, 384:640] = bm
    return {
        "blobA1": np.ascontiguousarray(blobA1),
        "blobA2": np.ascontiguousarray(blobA2),
        "rcb": rcb.astype(bf),
        "consts": consts,
        "wqT": np.ascontiguousarray(wqT),
        "adj": adj_blob.astype(bf),
        "blobB": np.ascontiguousarray(blobB),
    }


def _postprocess_core(out_dev, Af, fallback):
    """out_dev [128, 768] -> mapped [8, 48, 256]; apply fallback select."""
    arr = out_dev.reshape(128, 2, NB, N)
    mapped = np.ascontiguousarray(
        np.transpose(arr, (2, 3, 1, 0))).reshape(NB, N, FEA)
    cnt = Af.sum(axis=1)                              # [NB, 48(t)]
    return np.where((cnt > 0)[:, :, None], mapped, fallback)


def _make_in_maps(a):
    in_maps, meta = [], []
    for core in range(NCORES):
        dirn = "a" if core < 4 else "s"
        g = core % 4
        bs = slice(g * NB, (g + 1) * NB)
        if dirn == "a":
            x_src, adj, fb = a["sync_fea"][bs], a["sync_adj"][bs], a["async_fea"][bs]
        else:
            x_src, adj, fb = a["async_fea"][bs], a["async_adj"][bs], a["sync_fea"][bs]
        wkeys = [f"{dirn}_{w}" for w in
                 ("Wq", "bq", "Wk", "bk", "Wv", "bv", "Wo", "bo", "Wm", "bm")]
        in_maps.append(_prep_core_inputs(x_src, adj, *[a[k] for k in wkeys]))
        meta.append(((adj > 0).astype(np.float32), fb))
    return in_maps, meta


def _assemble(a, meta, results):
    out = np.zeros((B, N, 4 * FEA), np.float32)
    out[:, :, 2 * FEA:3 * FEA] = a["async_fea"]
    out[:, :, 3 * FEA:] = a["sync_fea"]
    for core in range(NCORES):
        Af, fb = meta[core]
        refined = _postprocess_core(results[core]["outT"], Af, fb)
        g = core % 4
        bs = slice(g * NB, (g + 1) * NB)
        col = slice(0, FEA) if core < 4 else slice(FEA, 2 * FEA)
        out[bs, :, col] = refined
    return out


def kernel(**inputs):
    from concourse import bass_utils

    nc = _get_program()
    a = {k: np.asarray(v) for k, v in inputs.items()}
    in_maps, meta = _make_in_maps(a)
    res = bass_utils.run_bass_kernel_spmd(nc, in_maps, core_ids=list(range(NCORES)))
    return _assemble(a, meta, res.results)


# revision 36
# speedup vs baseline: 1.0330x; 1.0330x over previous
"""Trainium2 Bass kernel for nn_Aggregate (gnn_message_passing).

Sharding: 8 cores = 2 directions x 4 batch-groups. Cores 0-3 compute
refined_async (source = sync_fea, adj = sync_adj, weights a_*) for 8
batches each; cores 4-7 compute refined_sync. The feature passthrough
(output channels 512:1024) and the no-neighbor fallback select are pure
input data movement, done host-side during unsharding.

Device algorithm per core (8 batches, one direction):
  Activations stay feature-major ([feat, node]); weights host-transposed
  to input-major (matmul lhsT). Batch parity j on partition halves
  (rows j*64+node).

    kT = WkT^T xT + bk ; qT = WqT^T xT + bq  (q pre-scaled 1/sqrt(dh))
    v  = xT^T WvT                            (node-major, bias folded out)
    per (head, batch):  Pq = exp(qT_h^T kT_h)   [q,k]
                        Pk = exp(kT_h^T qT_h)   [k,q]
                        den = Pk^T Af           [q,t]
                        w   = Af / den          [q,t]  (single divide)
                        ST  = Pq^T w            [k,t]
                        SmT = ST * (Af r^2)     [k,t]  (r = 1/cnt, host)
                        G_h = v_h^T SmT         [d,t]
    m2  = (WmWo)^T-contraction @ G  +  [c0|bm] rank-2 @ [r|1]
  The adj-derived normalizers cnt, r=1/max(cnt,1), r^2 are computed on
  host (adj is an input): r^2 rides in as Af*r^2 for the Sm mask-mult,
  and the affine tail c0*r + bm is a K=2 matmul accumulated into m2
  psum. c0 = Wm @ (Wo @ bv + bo): the v-bias folds out exactly because
  sum_k SmT_h[k,t] = cnt[t].

Performance structure (cost-model driven):
  - 3 parallel DMA queues (SP HWDGE / Pool SWDGE / Act HWDGE) so input
    loads don't serialize on the single HWDGE slot.
  - PE warmup matmuls during the load phase keep the Tensor engine
    continuously busy so its p-state ramp (3us to full clock) completes
    before real work; every real matmul then runs at max rate.
  - Scores psum is tiled per (orientation, feature-half) so exp and the
    den/S chain unblock per half; v matmuls interleave between den and
    S as PE filler while the divide chain runs on DVE; final copies are
    split across DVE and Act.

Built on bacc.Bacc: its compile() legalizes sync waits (TRN2 allows one
wait per instruction) via ldweights-wait motion + event semaphores.
"""

import numpy as np

FEA, H, B, N = 256, 8, 32, 48
DH = FEA // H
NB = 8            # batches per core
NPAIR = NB // 2
NCORES = 8
NT = NB * N       # 384

N_WARM = 66       # PE warmup matmuls (ap 64) covering the DMA load phase

_cached = None


def _build_program():
    import concourse.tile as tile
    from concourse.tile import add_dep_helper
    from concourse import bacc, mybir
    from contextlib import ExitStack

    f32 = mybir.dt.float32
    bf = mybir.dt.bfloat16
    AF = mybir.ActivationFunctionType
    OP = mybir.AluOpType

    nc = bacc.Bacc("TRN2", target_bir_lowering=False, debug=False)

    # ---- DRAM I/O ----
    # blobA1: xT both kc-chunks (768) | wk ot0 chunks (256) | bk,bq (4)
    blobA1_d = nc.dram_tensor("blobA1", [128, 1028], bf, kind="ExternalInput")
    blobA2_d = nc.dram_tensor("blobA2", [128, 256], bf, kind="ExternalInput")
    # rcb: [r|1] rows (384 cols) then [c0|bm] rows (256 cols)
    rcb_d = nc.dram_tensor("rcb", [2, 640], bf, kind="ExternalInput")
    # wq, ot-major chunks
    wq_d = nc.dram_tensor("wqT", [128, 512], bf, kind="ExternalInput")
    # adjst (192) | adjst * r^2 (192)
    adj_d = nc.dram_tensor("adj", [128, 384], bf, kind="ExternalInput")
    # blobB: wvT (512) | womT (512)
    blobB_d = nc.dram_tensor("blobB", [128, 1024], bf, kind="ExternalInput")
    out_d = nc.dram_tensor("outT", [128, 2 * NT], bf, kind="ExternalOutput")

    def p_off(pr, h):
        """Pq/Pk column of (pr, head) block: (ot, g//2, g%2, pr, t)."""
        ot, g = h // 4, h % 4
        return ot * 768 + (g // 2) * 384 + (g % 2) * 192 + pr * N

    with ExitStack() as ctx:
        tc = ctx.enter_context(tile.TileContext(nc))
        sb = ctx.enter_context(tc.tile_pool(name="sb", bufs=1))
        wm = ctx.enter_context(tc.tile_pool(name="wm", bufs=1, space="PSUM"))
        ps1 = ctx.enter_context(tc.tile_pool(name="ps1", bufs=3, space="PSUM"))
        ps2 = ctx.enter_context(tc.tile_pool(name="ps2", bufs=2, space="PSUM"))

        # ---- loads on 3 parallel queues ----
        blobA1 = sb.tile([128, 1028], bf, tag="blobA1")
        nc.sync.dma_start(out=blobA1[:, :], in_=blobA1_d.ap()[:, :])
        blobA2 = sb.tile([128, 256], bf, tag="blobA2")
        nc.sync.dma_start(out=blobA2[:, :], in_=blobA2_d.ap()[:, :])
        rcb = sb.tile([2, 640], bf, tag="rcb")
        nc.sync.dma_start(out=rcb[:, :], in_=rcb_d.ap()[:, :])
        wqT = sb.tile([128, 512], bf, tag="wqT")
        nc.scalar.dma_start(out=wqT[:, :], in_=wq_d.ap()[:, :])
        adj = sb.tile([128, 384], bf, tag="adj")
        nc.gpsimd.dma_start(out=adj[:, :], in_=adj_d.ap()[:, :])
        blobB = sb.tile([128, 1024], bf, tag="blobB")
        nc.scalar.dma_start(out=blobB[:, :], in_=blobB_d.ap()[:, :])

        xT = [blobA1[:, kc * 384:(kc + 1) * 384] for kc in range(2)]

        def wk_sl(ot, kc):
            if ot == 0:
                return blobA1[:, 768 + kc * 128:768 + (kc + 1) * 128]
            return blobA2[:, kc * 128:(kc + 1) * 128]

        def wq_sl(ot, kc):
            return wqT[:, ot * 256 + kc * 128:ot * 256 + (kc + 1) * 128]

        adjst = adj[:, 0:192]
        adjst2 = adj[:, 192:384]
        wvT = blobB[:, 0:512]
        womT = blobB[:, 512:1024]
        bk, bq = blobA1[:, 1024:1026], blobA1[:, 1026:1028]
        rcx = rcb[:, 0:NT]            # rows: [r | ones]
        rcw = rcb[:, 384:640]         # rows: [c0 | bm]

        ones = sb.tile([N, 128], bf, tag="ones")
        nc.vector.memset(ones[:, :], 1.0)

        # ---- PE warmup: keep Tensor busy through the load phase so the
        # p-state ramp finishes before real matmuls. Results unread. ----
        cp = wm.tile([128, 512], f32, tag="cp")
        for i in range(N_WARM):
            nc.tensor.matmul(cp[0:64, 0:64], ones[:, 0:64], ones[:, 0:64],
                             start=True, stop=True)

        # ---- k/q projections (feature-major), psum tile per ot-half;
        # ot0 pair first so their bias TTs and scores unblock earliest.
        # All staged SBUF tensors are split per half/pair: Tile tracks
        # dependencies at tile granularity, so fine tiles keep readers of
        # one half from serializing behind writers of the other. ----
        kqps = {}
        for ot in range(2):
            p = ps2.tile([128, 2, 512], f32, tag="ps2t")
            for bank, (nm, w_sl) in enumerate((("k", wk_sl), ("q", wq_sl))):
                kqps[nm, ot] = p[:, bank, 0:NT]
                for kc in range(2):
                    nc.tensor.matmul(
                        p[:, bank, 0:NT], w_sl(ot, kc), xT[kc][:, :],
                        start=(kc == 0), stop=(kc == 1),
                    )

        # bias adds: ot0 pair on DVE, ot1 pair on Act (parallel chains)
        kqT = {}
        for ot in range(2):
            for nm, bias in (("k", bk), ("q", bq)):
                t = sb.tile([128, NT], bf, tag=f"{nm}T{ot}")
                kqT[nm, ot] = t
                if ot == 0:
                    nc.vector.tensor_tensor(
                        out=t[:, :], in0=kqps[nm, ot],
                        in1=bias[:, 0:1].to_broadcast((128, NT)),
                        op=OP.add,
                    )
                else:
                    nc.scalar.activation(
                        out=t[:, :], in_=kqps[nm, ot], func=AF.Identity,
                        bias=bias[:, 1:2],
                    )

        def head_slice(nm, h, b):
            """[32, 48] lhsT/rhs slice of the per-ot feature-major tile."""
            t = kqT[nm, h // 4]
            return t[(h % 4) * 32:(h % 4) * 32 + 32, b * N:(b + 1) * N]

        def serial_rowgroups(groups):
            """Same-bank psum row-group serialization (in-order, no-cost)."""
            for gi in range(1, len(groups)):
                for i1 in groups[gi]:
                    for i0 in groups[gi - 1]:
                        add_dep_helper(i1.ins, i0.ins, sync=True,
                                       reason="same-bank row-group serialization")

        # ---- scores + exp, per (orientation, feature-half ot) ----
        # psum tile per (orient, ot): bank b holds heads g=2b,2b+1 (PE
        # row-tiles g*32), cols (g%2)*192 + pr*48. exp -> P[orient][ot].
        P = {o: [None, None] for o in ("k", "q")}

        def po(pr, h):
            g = h % 4
            return (g // 2) * 384 + (g % 2) * 192 + pr * N

        def scores_phase(orient, ot):
            lhs, rhs = ("k", "q") if orient == "k" else ("q", "k")
            p = ps2.tile([128, 2, 512], f32, tag="ps2t")
            groups = []
            for g in range(4):
                h = ot * 4 + g
                grp = []
                for j in range(2):
                    for pr in range(NPAIR):
                        b = pr * 2 + j
                        grp.append(nc.tensor.matmul(
                            p[j * 64:j * 64 + N, g // 2,
                              (g % 2) * 192 + pr * N:(g % 2) * 192 + (pr + 1) * N],
                            head_slice(lhs, h, b),
                            head_slice(rhs, h, b),
                            start=True, stop=True,
                            tile_position=(g * 32, j * 64),
                        ))
                groups.append(grp)
            # A psum bank may not take concurrent writes from different
            # PE row-tiles: bank g//2 receives row-tiles g*32 for both
            # g's of its pair, so serialize g=2b+1 after g=2b per bank.
            serial_rowgroups([groups[0], groups[1]])
            serial_rowgroups([groups[2], groups[3]])
            dst = sb.tile([128, 768], bf, tag=f"P{orient}{ot}")
            P[orient][ot] = dst
            nc.scalar.activation(
                out=dst[:, :].rearrange("p (b f) -> p b f", f=384),
                in_=p[:, :, 0:384], func=AF.Exp,
            )

        # ---- den -> w (= Af/den, one divide per pair) ----
        wT = {}

        def den_phase(pr):
            dp = ps1.tile([128, 512], f32, tag="ps1t")
            groups = [[], []]
            for j in range(2):
                for h in range(H):
                    groups[j].append(nc.tensor.matmul(
                        dp[j * 64:j * 64 + N, h * N:(h + 1) * N],
                        P["k"][h // 4][j * 64:j * 64 + N,
                                       po(pr, h):po(pr, h) + N],
                        adjst[j * 64:j * 64 + N, pr * N:(pr + 1) * N],
                        start=True, stop=True,
                    ))
            serial_rowgroups(groups)
            wt = sb.tile([128, 384], bf, tag=f"wT{pr}")
            wT[pr] = wt
            eng = nc.vector if pr % 2 == 0 else nc.gpsimd
            with nc.allow_low_precision(reason="bf16 attn weights; accum fp32"):
                eng.tensor_tensor(
                    out=wt[:, :].rearrange("p (h t) -> p h t", t=N),
                    in0=adjst[:, pr * N:(pr + 1) * N][:, None, :]
                        .to_broadcast((128, H, N)),
                    in1=dp[:, 0:384].rearrange("p (h t) -> p h t", t=N),
                    op=OP.divide,
                )

        # ---- v (node-major: rows j*64+n, half = pr//2), reusing the
        # warmup psum bank half-by-half; copies split Pool / Act ----
        v = [sb.tile([128, 2, 256], bf, tag=f"v{half}", name=f"v{half}")
             for half in range(2)]

        def v_mms(half):
            for bb in range(4 * half, 4 * half + 4):
                pr, j = bb // 2, bb % 2
                for kc in range(2):
                    nc.tensor.matmul(
                        cp[j * 64:j * 64 + N,
                           (pr % 2) * 256:(pr % 2 + 1) * 256],
                        xT[kc][:, bb * N:(bb + 1) * N],
                        wvT[:, kc * 256:(kc + 1) * 256],
                        start=(kc == 0), stop=(kc == 1),
                    )
            if half == 0:
                nc.gpsimd.tensor_copy(
                    out=v[half][:, :, :],
                    in_=cp[:, :].rearrange("p (c o) -> p c o", o=256),
                )
            else:
                nc.scalar.activation(
                    out=v[half][:, :, :],
                    in_=cp[:, :].rearrange("p (c o) -> p c o", o=256),
                    func=AF.Copy,
                )

        # ---- S -> Sm per (pair, head-half): the h0-3 half only needs
        # Pq-ot0, so G bank 0 decouples from the last exp ----
        SmT = {}

        def s_phase(pr):
            sp = ps1.tile([128, 512], f32, tag="ps1t")
            groups = []
            for hf in range(2):
                for j in range(2):
                    grp = []
                    for h in range(hf * 4, hf * 4 + 4):
                        grp.append(nc.tensor.matmul(
                            sp[j * 64:j * 64 + N, h * N:(h + 1) * N],
                            P["q"][hf][j * 64:j * 64 + N,
                                       po(pr, h):po(pr, h) + N],
                            wT[pr][j * 64:j * 64 + N, h * N:h * N + N],
                            start=True, stop=True,
                        ))
                    groups.append(grp)
                smt = sb.tile([128, 4, N], bf, tag=f"SmT{pr}{hf}")
                SmT[pr, hf] = smt
                # Sm = S * (Af r^2): r^2 pooling scale folded into the mask
                eng = nc.vector if hf == 0 else nc.gpsimd
                eng.tensor_tensor(
                    out=smt[:, :, :],
                    in0=sp[:, hf * 192:(hf + 1) * 192]
                        .rearrange("p (h t) -> p h t", t=N),
                    in1=adjst2[:, pr * N:(pr + 1) * N][:, None, :]
                        .to_broadcast((128, 4, N)),
                    op=OP.mult,
                )
            serial_rowgroups(groups)

        # scores-q-ot1 sits between v halves: its psum slot frees via
        # exp-k-ot1 and its Pq half is only consumed by the S h4-7 halves.
        scores_phase("k", 0)
        scores_phase("k", 1)
        scores_phase("q", 0)
        for pr in range(NPAIR):
            den_phase(pr)
        v_mms(0)
        scores_phase("q", 1)
        s_phase(0)
        s_phase(1)
        v_mms(1)
        s_phase(2)
        s_phase(3)

        # ---- G: pooled, feature-major; bank b = h//4 (= kc of Wm@Wo) in
        # its own psum tile so the bank-0 copy starts as soon as its own
        # matmuls finish; cols (pr, j, t) = output order ----
        Gs = [sb.tile([128, NT], bf, tag=f"Gs{b}", name=f"Gs{b}")
              for b in range(2)]
        for bank in range(2):
            gp = ps1.tile([128, 512], f32, tag="ps1t")
            groups = [[], []]
            for j in range(2):
                for pr in range(NPAIR):
                    for hh in range(4):
                        h = bank * 4 + hh
                        groups[j].append(nc.tensor.matmul(
                            gp[hh * 32:hh * 32 + 32,
                               pr * 96 + j * 48:pr * 96 + j * 48 + N],
                            v[pr // 2][j * 64:j * 64 + N, pr % 2,
                                       h * 32:(h + 1) * 32],
                            SmT[pr, bank][j * 64:j * 64 + N, hh, :],
                            start=True, stop=True,
                            tile_position=(j * 64, hh * 32),
                        ))
            serial_rowgroups(groups)
            nc.vector.tensor_copy(out=Gs[bank][:, :], in_=gp[:, 0:NT])

        # ---- m2 = (WmWo)^T-contraction @ Gs + rank-2 affine (c0*r + bm);
        # per-ot psum tiles so ot1's matmuls don't wait on ot0's copy ----
        m2 = [ps1.tile([128, 512], f32, tag="ps1t", name=f"m2_{i}")
              for i in range(2)]
        for kc in range(2):
            for ot in range(2):
                nc.tensor.matmul(
                    m2[ot][:, 0:NT],
                    womT[:, kc * 256 + ot * 128:kc * 256 + (ot + 1) * 128],
                    Gs[kc][:, :],
                    start=(kc == 0), stop=False,
                )
        for ot in range(2):
            nc.tensor.matmul(
                m2[ot][:, 0:NT],
                rcw[:, ot * 128:(ot + 1) * 128],
                rcx[:, :],
                start=False, stop=True,
            )
        osb0 = sb.tile([128, NT], bf, tag="osb0")
        nc.vector.tensor_copy(out=osb0[:, :], in_=m2[0][:, 0:NT])
        nc.sync.dma_start(out=out_d.ap()[:, 0:NT], in_=osb0[:, :])
        osb1 = sb.tile([128, NT], bf, tag="osb1")
        nc.scalar.activation(out=osb1[:, :], in_=m2[1][:, 0:NT], func=AF.Copy)
        nc.scalar.dma_start(out=out_d.ap()[:, NT:2 * NT], in_=osb1[:, :])

    nc.compile()
    return nc


def _get_program():
    global _cached
    if _cached is None:
        _cached = _build_program()
    return _cached


def _prep_core_inputs(x_src, adj, Wq, bq, Wk, bk, Wv, bv, Wo, bo, Wm, bm):
    """Host-side shard prep for one core: 8 batches of one direction.
    Matmul-side tensors are cast to bfloat16 (PSUM accumulation stays fp32;
    the reference's own fp32 noise dominates the resulting error)."""
    import ml_dtypes
    f32 = np.float32
    bf = ml_dtypes.bfloat16
    xT = np.ascontiguousarray(
        np.transpose(x_src, (2, 0, 1)).reshape(FEA, NT)).astype(bf)
    Af = (adj > 0).astype(f32)                       # [NB, 48(k), 48(t)]
    s = 1.0 / np.sqrt(np.float32(DH))

    def ot_chunks(w):   # [256(in), 256(out)] W.T -> [128, 512] (ot, kc)
        wt = np.ascontiguousarray(w)
        return np.concatenate([wt[kc * 128:(kc + 1) * 128, ot * 128:(ot + 1) * 128]
                               for ot in range(2) for kc in range(2)], axis=1)

    def kc_chunks(w):   # [256, 256] W.T -> [128, 512] (kc major, full out)
        wt = np.ascontiguousarray(w)
        return np.concatenate([wt[0:128, :], wt[128:256, :]], axis=1)

    wkT = ot_chunks(Wk.T).astype(bf)
    consts = np.zeros((128, 4), f32)
    consts[:, 0:2] = bk.reshape(2, 128).T
    consts[:, 2:4] = (bq * s).reshape(2, 128).T
    blobA1 = np.concatenate(
        [xT[0:128, :], xT[128:256, :], wkT[:, 0:256], consts.astype(bf)], axis=1)
    blobA2 = wkT[:, 256:512]
    wqT = ot_chunks(Wq.T * s).astype(bf)
    blobB = np.concatenate(
        [kc_chunks(Wv.T).astype(bf), kc_chunks((Wm @ Wo).T).astype(bf)], axis=1)

    cnt = Af.sum(axis=1)                             # [NB, 48(t)]
    r = (1.0 / np.maximum(cnt, 1.0)).astype(f32)     # [NB, 48]
    r2 = r * r

    adj_blob = np.zeros((128, 384), f32)
    for p in range(NPAIR):
        adj_blob[0:N, p * N:(p + 1) * N] = Af[2 * p]
        adj_blob[64:64 + N, p * N:(p + 1) * N] = Af[2 * p + 1]
        adj_blob[0:N, 192 + p * N:192 + (p + 1) * N] = Af[2 * p] * r2[2 * p]
        adj_blob[64:64 + N, 192 + p * N:192 + (p + 1) * N] = \
            Af[2 * p + 1] * r2[2 * p + 1]

    c0 = (Wm @ (Wo @ bv + bo)).astype(f32)

    rcb = np.zeros((2, 640), f32)
    rcb[0, 0:NT] = r.reshape(NT)
    rcb[1, 0:NT] = 1.0
    rcb[0, 384:640] = c0
    rcb[1, 384:640] = bm
    return {
        "blobA1": np.ascontiguousarray(blobA1),
        "blobA2": np.ascontiguousarray(blobA2),
        "rcb": rcb.astype(bf),
        "wqT": np.ascontiguousarray(wqT),
        "adj": adj_blob.astype(bf),
        "blobB": np.ascontiguousarray(blobB),
    }


def _postprocess_core(out_dev, Af, fallback):
    """out_dev [128, 768] -> mapped [8, 48, 256]; apply fallback select."""
    arr = out_dev.reshape(128, 2, NB, N)
    mapped = np.ascontiguousarray(
        np.transpose(arr, (2, 3, 1, 0))).reshape(NB, N, FEA)
    cnt = Af.sum(axis=1)                              # [NB, 48(t)]
    return np.where((cnt > 0)[:, :, None], mapped, fallback)


def _make_in_maps(a):
    in_maps, meta = [], []
    for core in range(NCORES):
        dirn = "a" if core < 4 else "s"
        g = core % 4
        bs = slice(g * NB, (g + 1) * NB)
        if dirn == "a":
            x_src, adj, fb = a["sync_fea"][bs], a["sync_adj"][bs], a["async_fea"][bs]
        else:
            x_src, adj, fb = a["async_fea"][bs], a["async_adj"][bs], a["sync_fea"][bs]
        wkeys = [f"{dirn}_{w}" for w in
                 ("Wq", "bq", "Wk", "bk", "Wv", "bv", "Wo", "bo", "Wm", "bm")]
        in_maps.append(_prep_core_inputs(x_src, adj, *[a[k] for k in wkeys]))
        meta.append(((adj > 0).astype(np.float32), fb))
    return in_maps, meta


def _assemble(a, meta, results):
    out = np.zeros((B, N, 4 * FEA), np.float32)
    out[:, :, 2 * FEA:3 * FEA] = a["async_fea"]
    out[:, :, 3 * FEA:] = a["sync_fea"]
    for core in range(NCORES):
        Af, fb = meta[core]
        refined = _postprocess_core(results[core]["outT"], Af, fb)
        g = core % 4
        bs = slice(g * NB, (g + 1) * NB)
        col = slice(0, FEA) if core < 4 else slice(FEA, 2 * FEA)
        out[bs, :, col] = refined
    return out


def kernel(**inputs):
    from concourse import bass_utils

    nc = _get_program()
    a = {k: np.asarray(v) for k, v in inputs.items()}
    in_maps, meta = _make_in_maps(a)
    res = bass_utils.run_bass_kernel_spmd(nc, in_maps, core_ids=list(range(NCORES)))
    return _assemble(a, meta, res.results)


# revision 40
# speedup vs baseline: 1.0741x; 1.0397x over previous
"""Trainium2 Bass kernel for nn_Aggregate (gnn_message_passing).

Sharding: 8 cores = 2 directions x 4 batch-groups. Cores 0-3 compute
refined_async (source = sync_fea, adj = sync_adj, weights a_*) for 8
batches each; cores 4-7 compute refined_sync. The feature passthrough
(output channels 512:1024) and the no-neighbor fallback select are pure
input data movement, done host-side during unsharding.

Device algorithm per core (8 batches, one direction):
  Activations stay feature-major ([feat, node]); weights host-transposed
  to input-major (matmul lhsT). Batch parity j on partition halves
  (rows j*64+node).

    kT = WkT^T xT + bk ; qT = WqT^T xT + bq  (q pre-scaled 1/sqrt(dh))
    v  = xT^T WvT                            (node-major, bias folded out)
    per (head, batch):  Pq = exp(qT_h^T kT_h)   [q,k]
                        Pk = exp(kT_h^T qT_h)   [k,q]
                        den = Pk^T Af           [q,t]
                        w   = Af / den          [q,t]  (single divide)
                        ST  = Pq^T w            [k,t]
                        SmT = ST * (Af r^2)     [k,t]  (r = 1/cnt, host)
                        G_h = v_h^T SmT         [d,t]
    m2  = (WmWo)^T-contraction @ G  +  [c0|bm] rank-2 @ [r|1]
  The adj-derived normalizers cnt, r=1/max(cnt,1), r^2 are computed on
  host (adj is an input): r^2 rides in as Af*r^2 for the Sm mask-mult,
  and the affine tail c0*r + bm is a K=2 matmul accumulated into m2
  psum. c0 = Wm @ (Wo @ bv + bo): the v-bias folds out exactly because
  sum_k SmT_h[k,t] = cnt[t].

Performance structure (cost-model driven):
  - 3 parallel DMA queues (SP HWDGE / Pool SWDGE / Act HWDGE) so input
    loads don't serialize on the single HWDGE slot.
  - PE warmup matmuls during the load phase keep the Tensor engine
    continuously busy so its p-state ramp (3us to full clock) completes
    before real work; every real matmul then runs at max rate.
  - Scores psum is tiled per (orientation, feature-half) so exp and the
    den/S chain unblock per half; v matmuls interleave between den and
    S as PE filler while the divide chain runs on DVE; final copies are
    split across DVE and Act.

Built on bacc.Bacc: its compile() legalizes sync waits (TRN2 allows one
wait per instruction) via ldweights-wait motion + event semaphores.
"""

import numpy as np

FEA, H, B, N = 256, 8, 32, 48
DH = FEA // H
NB = 8            # batches per core
NPAIR = NB // 2
NCORES = 8
NT = NB * N       # 384

N_WARM = 66       # PE warmup matmuls (ap 64) covering the DMA load phase

_cached = None


def _build_program():
    import concourse.tile as tile
    from concourse.tile import add_dep_helper
    from concourse import bacc, mybir
    from contextlib import ExitStack

    f32 = mybir.dt.float32
    bf = mybir.dt.bfloat16
    AF = mybir.ActivationFunctionType
    OP = mybir.AluOpType

    nc = bacc.Bacc("TRN2", target_bir_lowering=False, debug=False)

    # ---- DRAM I/O ----
    # blobA1: xT both kc-chunks (768) | wk ot0 chunks (256) | bk,bq (4)
    blobA1_d = nc.dram_tensor("blobA1", [128, 1028], bf, kind="ExternalInput")
    blobA2_d = nc.dram_tensor("blobA2", [128, 256], bf, kind="ExternalInput")
    # rcb: [r|1] rows (384 cols) then [c0|bm] rows (256 cols)
    rcb_d = nc.dram_tensor("rcb", [2, 640], bf, kind="ExternalInput")
    # wq, ot-major chunks
    wq_d = nc.dram_tensor("wqT", [128, 512], bf, kind="ExternalInput")
    # adjst (192) | adjst * r^2 (192)
    adj_d = nc.dram_tensor("adj", [128, 384], bf, kind="ExternalInput")
    # blobB: wvT (512) | womT (512)
    blobB_d = nc.dram_tensor("blobB", [128, 1024], bf, kind="ExternalInput")
    out_d = nc.dram_tensor("outT", [128, 2 * NT], bf, kind="ExternalOutput")

    def p_off(pr, h):
        """Pq/Pk column of (pr, head) block: (ot, g//2, g%2, pr, t)."""
        ot, g = h // 4, h % 4
        return ot * 768 + (g // 2) * 384 + (g % 2) * 192 + pr * N

    with ExitStack() as ctx:
        tc = ctx.enter_context(tile.TileContext(nc))
        sb = ctx.enter_context(tc.tile_pool(name="sb", bufs=1))
        wm = ctx.enter_context(tc.tile_pool(name="wm", bufs=1, space="PSUM"))
        ps1 = ctx.enter_context(tc.tile_pool(name="ps1", bufs=3, space="PSUM"))
        ps2 = ctx.enter_context(tc.tile_pool(name="ps2", bufs=2, space="PSUM"))

        # ---- loads on 3 parallel queues ----
        blobA1 = sb.tile([128, 1028], bf, tag="blobA1")
        nc.sync.dma_start(out=blobA1[:, :], in_=blobA1_d.ap()[:, :])
        blobA2 = sb.tile([128, 256], bf, tag="blobA2")
        nc.sync.dma_start(out=blobA2[:, :], in_=blobA2_d.ap()[:, :])
        rcb = sb.tile([2, 640], bf, tag="rcb")
        nc.sync.dma_start(out=rcb[:, :], in_=rcb_d.ap()[:, :])
        wqT = sb.tile([128, 512], bf, tag="wqT")
        nc.scalar.dma_start(out=wqT[:, :], in_=wq_d.ap()[:, :])
        adj = sb.tile([128, 384], bf, tag="adj")
        nc.gpsimd.dma_start(out=adj[:, :], in_=adj_d.ap()[:, :])
        blobB = sb.tile([128, 1024], bf, tag="blobB")
        nc.scalar.dma_start(out=blobB[:, :], in_=blobB_d.ap()[:, :])

        xT = [blobA1[:, kc * 384:(kc + 1) * 384] for kc in range(2)]

        def wk_sl(ot, kc):
            if ot == 0:
                return blobA1[:, 768 + kc * 128:768 + (kc + 1) * 128]
            return blobA2[:, kc * 128:(kc + 1) * 128]

        def wq_sl(ot, kc):
            return wqT[:, ot * 256 + kc * 128:ot * 256 + (kc + 1) * 128]

        adjst = adj[:, 0:192]
        adjst2 = adj[:, 192:384]
        wvT = blobB[:, 0:512]
        womT = blobB[:, 512:1024]
        bk, bq = blobA1[:, 1024:1026], blobA1[:, 1026:1028]
        rcx = rcb[:, 0:NT]            # rows: [r | ones]
        rcw = rcb[:, 384:640]         # rows: [c0 | bm]

        ones = sb.tile([N, 128], bf, tag="ones")
        nc.vector.memset(ones[:, :], 1.0)

        # Explicit per-engine issue-order chains: the Tile scheduler
        # orders each engine stream with a legacy cost model whose DMA /
        # latency estimates diverge from the timeline model, so pin the
        # orders derived from the critical-path analysis instead.
        chain = {"PE": [], "DVE": [], "ACT": [], "POOL": []}

        def ch(eng, instr):
            chain[eng].append(instr)
            return instr

        # ---- PE warmup: keep Tensor busy through the load phase so the
        # p-state ramp finishes before real matmuls. Results unread. ----
        cp = wm.tile([128, 512], f32, tag="cp")
        for i in range(N_WARM):
            ch("PE", nc.tensor.matmul(cp[0:64, 0:64], ones[:, 0:64],
                                      ones[:, 0:64], start=True, stop=True))

        # ---- k/q projections (feature-major), psum tile per ot-half;
        # ot0 pair first so their bias TTs and scores unblock earliest.
        # All staged SBUF tensors are split per half/pair: Tile tracks
        # dependencies at tile granularity, so fine tiles keep readers of
        # one half from serializing behind writers of the other. ----
        kqps = {}
        for ot in range(2):
            p = ps2.tile([128, 2, 512], f32, tag="ps2t")
            for bank, (nm, w_sl) in enumerate((("k", wk_sl), ("q", wq_sl))):
                kqps[nm, ot] = p[:, bank, 0:NT]
                for kc in range(2):
                    ch("PE", nc.tensor.matmul(
                        p[:, bank, 0:NT], w_sl(ot, kc), xT[kc][:, :],
                        start=(kc == 0), stop=(kc == 1),
                    ))

        # bias adds: ot0 pair on DVE, ot1 pair on Act (parallel chains)
        kqT = {}
        for ot in range(2):
            for nm, bias in (("k", bk), ("q", bq)):
                t = sb.tile([128, NT], bf, tag=f"{nm}T{ot}")
                kqT[nm, ot] = t
                if ot == 0:
                    ch("DVE", nc.vector.tensor_tensor(
                        out=t[:, :], in0=kqps[nm, ot],
                        in1=bias[:, 0:1].to_broadcast((128, NT)),
                        op=OP.add,
                    ))
                else:
                    ch("ACT", nc.scalar.activation(
                        out=t[:, :], in_=kqps[nm, ot], func=AF.Identity,
                        bias=bias[:, 1:2],
                    ))

        def head_slice(nm, h, b):
            """[32, 48] lhsT/rhs slice of the per-ot feature-major tile."""
            t = kqT[nm, h // 4]
            return t[(h % 4) * 32:(h % 4) * 32 + 32, b * N:(b + 1) * N]

        def serial_rowgroups(groups):
            """Same-bank psum row-group serialization (in-order, no-cost)."""
            for gi in range(1, len(groups)):
                for i1 in groups[gi]:
                    for i0 in groups[gi - 1]:
                        add_dep_helper(i1.ins, i0.ins, sync=True,
                                       reason="same-bank row-group serialization")

        # ---- scores + exp, per (orientation, feature-half ot) ----
        # psum tile per (orient, ot): bank b holds heads g=2b,2b+1 (PE
        # row-tiles g*32), cols (g%2)*192 + pr*48. exp -> P[orient][ot].
        P = {o: [None, None] for o in ("k", "q")}

        def po(pr, h):
            g = h % 4
            return (g // 2) * 384 + (g % 2) * 192 + pr * N

        def scores_phase(orient, ot):
            lhs, rhs = ("k", "q") if orient == "k" else ("q", "k")
            p = ps2.tile([128, 2, 512], f32, tag="ps2t")
            groups = []
            for g in range(4):
                h = ot * 4 + g
                grp = []
                for j in range(2):
                    for pr in range(NPAIR):
                        b = pr * 2 + j
                        grp.append(ch("PE", nc.tensor.matmul(
                            p[j * 64:j * 64 + N, g // 2,
                              (g % 2) * 192 + pr * N:(g % 2) * 192 + (pr + 1) * N],
                            head_slice(lhs, h, b),
                            head_slice(rhs, h, b),
                            start=True, stop=True,
                            tile_position=(g * 32, j * 64),
                        )))
                groups.append(grp)
            serial_rowgroups([groups[0], groups[1]])
            serial_rowgroups([groups[2], groups[3]])
            dst = sb.tile([128, 768], bf, tag=f"P{orient}{ot}")
            P[orient][ot] = dst
            ch("ACT", nc.scalar.activation(
                out=dst[:, :].rearrange("p (b f) -> p b f", f=384),
                in_=p[:, :, 0:384], func=AF.Exp,
            ))

        # ---- den -> w (= Af/den, one divide per pair) ----
        wT = {}

        def den_phase(pr):
            dp = ps1.tile([128, 512], f32, tag="ps1t")
            groups = [[], []]
            for j in range(2):
                for h in range(H):
                    groups[j].append(ch("PE", nc.tensor.matmul(
                        dp[j * 64:j * 64 + N, h * N:(h + 1) * N],
                        P["k"][h // 4][j * 64:j * 64 + N,
                                       po(pr, h):po(pr, h) + N],
                        adjst[j * 64:j * 64 + N, pr * N:(pr + 1) * N],
                        start=True, stop=True,
                    )))
            serial_rowgroups(groups)
            wt = sb.tile([128, 384], bf, tag=f"wT{pr}")
            wT[pr] = wt
            eng = ("DVE", nc.vector) if pr % 2 == 0 else ("POOL", nc.gpsimd)
            with nc.allow_low_precision(reason="bf16 attn weights; accum fp32"):
                ch(eng[0], eng[1].tensor_tensor(
                    out=wt[:, :].rearrange("p (h t) -> p h t", t=N),
                    in0=adjst[:, pr * N:(pr + 1) * N][:, None, :]
                        .to_broadcast((128, H, N)),
                    in1=dp[:, 0:384].rearrange("p (h t) -> p h t", t=N),
                    op=OP.divide,
                ))

        # ---- v (node-major: rows j*64+n, half = pr//2), reusing the
        # warmup psum bank half-by-half; copies split Pool / Act ----
        v = [sb.tile([128, 2, 256], bf, tag=f"v{half}", name=f"v{half}")
             for half in range(2)]

        def v_mms(half):
            for bb in range(4 * half, 4 * half + 4):
                pr, j = bb // 2, bb % 2
                for kc in range(2):
                    ch("PE", nc.tensor.matmul(
                        cp[j * 64:j * 64 + N,
                           (pr % 2) * 256:(pr % 2 + 1) * 256],
                        xT[kc][:, bb * N:(bb + 1) * N],
                        wvT[:, kc * 256:(kc + 1) * 256],
                        start=(kc == 0), stop=(kc == 1),
                    ))
            if half == 0:
                ch("POOL", nc.gpsimd.tensor_copy(
                    out=v[half][:, :, :],
                    in_=cp[:, :].rearrange("p (c o) -> p c o", o=256),
                ))
            else:
                ch("ACT", nc.scalar.activation(
                    out=v[half][:, :, :],
                    in_=cp[:, :].rearrange("p (c o) -> p c o", o=256),
                    func=AF.Copy,
                ))

        # ---- S -> Sm per (pair, head-half): all h0-3 halves (needing
        # only Pq-ot0) run before any h4-7 half, so G bank 0 and the Sm-a
        # chain decouple from the last exp ----
        SmT = {}

        def s_half(pr, hf):
            # own psum tile per (pair, half): frees via its single Sm TT,
            # so the pinned all-h03-then-h47 order cannot deadlock slots
            sp = ps1.tile([128, 512], f32, tag="ps1t", name=f"sS{pr}_{hf}")
            groups = [[], []]
            for j in range(2):
                for h in range(hf * 4, hf * 4 + 4):
                    groups[j].append(ch("PE", nc.tensor.matmul(
                        sp[j * 64:j * 64 + N,
                           (h - hf * 4) * N:(h - hf * 4 + 1) * N],
                        P["q"][hf][j * 64:j * 64 + N,
                                   po(pr, h):po(pr, h) + N],
                        wT[pr][j * 64:j * 64 + N, h * N:h * N + N],
                        start=True, stop=True,
                    )))
            serial_rowgroups(groups)
            smt = sb.tile([128, 4, N], bf, tag=f"SmT{pr}{hf}")
            SmT[pr, hf] = smt
            # Sm = S * (Af r^2): r^2 pooling scale folded into the mask.
            # a-halves on DVE; b-halves alternate Pool/DVE.
            eng = ("DVE", nc.vector) if (hf == 0 or pr % 2 == 1) \
                else ("POOL", nc.gpsimd)
            ch(eng[0], eng[1].tensor_tensor(
                out=smt[:, :, :],
                in0=sp[:, 0:192].rearrange("p (h t) -> p h t", t=N),
                in1=adjst2[:, pr * N:(pr + 1) * N][:, None, :]
                    .to_broadcast((128, 4, N)),
                op=OP.mult,
            ))

        # ---- emission in pinned order ----
        scores_phase("k", 0)
        scores_phase("k", 1)
        v_mms(0)
        scores_phase("q", 0)
        for pr in range(NPAIR):
            den_phase(pr)
        scores_phase("q", 1)
        s_half(0, 0)
        v_mms(1)
        s_half(1, 0)
        s_half(2, 0)
        s_half(3, 0)
        for pr in range(NPAIR):
            s_half(pr, 1)

        # ---- G: pooled, feature-major; bank b = h//4 (= kc of Wm@Wo) in
        # its own psum tile; cols (pr, j, t) = output order ----
        Gs = [sb.tile([128, NT], bf, tag=f"Gs{b}", name=f"Gs{b}")
              for b in range(2)]
        for bank in range(2):
            gp = ps1.tile([128, 512], f32, tag="ps1t")
            groups = [[], []]
            for j in range(2):
                for pr in range(NPAIR):
                    for hh in range(4):
                        h = bank * 4 + hh
                        groups[j].append(ch("PE", nc.tensor.matmul(
                            gp[hh * 32:hh * 32 + 32,
                               pr * 96 + j * 48:pr * 96 + j * 48 + N],
                            v[pr // 2][j * 64:j * 64 + N, pr % 2,
                                       h * 32:(h + 1) * 32],
                            SmT[pr, bank][j * 64:j * 64 + N, hh, :],
                            start=True, stop=True,
                            tile_position=(j * 64, hh * 32),
                        )))
            serial_rowgroups(groups)
            ch("DVE", nc.vector.tensor_copy(out=Gs[bank][:, :],
                                            in_=gp[:, 0:NT]))

        # ---- m2 = (WmWo)^T-contraction @ Gs + rank-2 affine (c0*r + bm);
        # per-ot psum tiles so ot1's matmuls don't wait on ot0's copy ----
        m2 = [ps1.tile([128, 512], f32, tag="ps1t", name=f"m2_{i}")
              for i in range(2)]
        for kc in range(2):
            for ot in range(2):
                ch("PE", nc.tensor.matmul(
                    m2[ot][:, 0:NT],
                    womT[:, kc * 256 + ot * 128:kc * 256 + (ot + 1) * 128],
                    Gs[kc][:, :],
                    start=(kc == 0), stop=False,
                ))
        for ot in range(2):
            ch("PE", nc.tensor.matmul(
                m2[ot][:, 0:NT],
                rcw[:, ot * 128:(ot + 1) * 128],
                rcx[:, :],
                start=False, stop=True,
            ))
        osb0 = sb.tile([128, NT], bf, tag="osb0")
        ch("DVE", nc.vector.tensor_copy(out=osb0[:, :], in_=m2[0][:, 0:NT]))
        nc.sync.dma_start(out=out_d.ap()[:, 0:NT], in_=osb0[:, :])
        osb1 = sb.tile([128, NT], bf, tag="osb1")
        ch("ACT", nc.scalar.activation(out=osb1[:, :], in_=m2[1][:, 0:NT],
                                       func=AF.Copy))
        nc.scalar.dma_start(out=out_d.ap()[:, NT:2 * NT], in_=osb1[:, :])

        # pin each engine's issue order
        for eng, instrs in chain.items():
            for i1, i0 in zip(instrs[1:], instrs):
                add_dep_helper(i1.ins, i0.ins, sync=True,
                               reason=f"{eng} issue-order pin")

    nc.compile()
    return nc


def _get_program():
    global _cached
    if _cached is None:
        _cached = _build_program()
    return _cached


def _prep_core_inputs(x_src, adj, Wq, bq, Wk, bk, Wv, bv, Wo, bo, Wm, bm):
    """Host-side shard prep for one core: 8 batches of one direction.
    Matmul-side tensors are cast to bfloat16 (PSUM accumulation stays fp32;
    the reference's own fp32 noise dominates the resulting error)."""
    import ml_dtypes
    f32 = np.float32
    bf = ml_dtypes.bfloat16
    xT = np.ascontiguousarray(
        np.transpose(x_src, (2, 0, 1)).reshape(FEA, NT)).astype(bf)
    Af = (adj > 0).astype(f32)                       # [NB, 48(k), 48(t)]
    s = 1.0 / np.sqrt(np.float32(DH))

    def ot_chunks(w):   # [256(in), 256(out)] W.T -> [128, 512] (ot, kc)
        wt = np.ascontiguousarray(w)
        return np.concatenate([wt[kc * 128:(kc + 1) * 128, ot * 128:(ot + 1) * 128]
                               for ot in range(2) for kc in range(2)], axis=1)

    def kc_chunks(w):   # [256, 256] W.T -> [128, 512] (kc major, full out)
        wt = np.ascontiguousarray(w)
        return np.concatenate([wt[0:128, :], wt[128:256, :]], axis=1)

    wkT = ot_chunks(Wk.T).astype(bf)
    consts = np.zeros((128, 4), f32)
    consts[:, 0:2] = bk.reshape(2, 128).T
    consts[:, 2:4] = (bq * s).reshape(2, 128).T
    blobA1 = np.concatenate(
        [xT[0:128, :], xT[128:256, :], wkT[:, 0:256], consts.astype(bf)], axis=1)
    blobA2 = wkT[:, 256:512]
    wqT = ot_chunks(Wq.T * s).astype(bf)
    blobB = np.concatenate(
        [kc_chunks(Wv.T).astype(bf), kc_chunks((Wm @ Wo).T).astype(bf)], axis=1)

    cnt = Af.sum(axis=1)                             # [NB, 48(t)]
    r = (1.0 / np.maximum(cnt, 1.0)).astype(f32)     # [NB, 48]
    r2 = r * r

    adj_blob = np.zeros((128, 384), f32)
    for p in range(NPAIR):
        adj_blob[0:N, p * N:(p + 1) * N] = Af[2 * p]
        adj_blob[64:64 + N, p * N:(p + 1) * N] = Af[2 * p + 1]
        adj_blob[0:N, 192 + p * N:192 + (p + 1) * N] = Af[2 * p] * r2[2 * p]
        adj_blob[64:64 + N, 192 + p * N:192 + (p + 1) * N] = \
            Af[2 * p + 1] * r2[2 * p + 1]

    c0 = (Wm @ (Wo @ bv + bo)).astype(f32)

    rcb = np.zeros((2, 640), f32)
    rcb[0, 0:NT] = r.reshape(NT)
    rcb[1, 0:NT] = 1.0
    rcb[0, 384:640] = c0
    rcb[1, 384:640] = bm
    return {
        "blobA1": np.ascontiguousarray(blobA1),
        "blobA2": np.ascontiguousarray(blobA2),
        "rcb": rcb.astype(bf),
        "wqT": np.ascontiguousarray(wqT),
        "adj": adj_blob.astype(bf),
        "blobB": np.ascontiguousarray(blobB),
    }


def _postprocess_core(out_dev, Af, fallback):
    """out_dev [128, 768] -> mapped [8, 48, 256]; apply fallback select."""
    arr = out_dev.reshape(128, 2, NB, N)
    mapped = np.ascontiguousarray(
        np.transpose(arr, (2, 3, 1, 0))).reshape(NB, N, FEA)
    cnt = Af.sum(axis=1)                              # [NB, 48(t)]
    return np.where((cnt > 0)[:, :, None], mapped, fallback)


def _make_in_maps(a):
    in_maps, meta = [], []
    for core in range(NCORES):
        dirn = "a" if core < 4 else "s"
        g = core % 4
        bs = slice(g * NB, (g + 1) * NB)
        if dirn == "a":
            x_src, adj, fb = a["sync_fea"][bs], a["sync_adj"][bs], a["async_fea"][bs]
        else:
            x_src, adj, fb = a["async_fea"][bs], a["async_adj"][bs], a["sync_fea"][bs]
        wkeys = [f"{dirn}_{w}" for w in
                 ("Wq", "bq", "Wk", "bk", "Wv", "bv", "Wo", "bo", "Wm", "bm")]
        in_maps.append(_prep_core_inputs(x_src, adj, *[a[k] for k in wkeys]))
        meta.append(((adj > 0).astype(np.float32), fb))
    return in_maps, meta


def _assemble(a, meta, results):
    out = np.zeros((B, N, 4 * FEA), np.float32)
    out[:, :, 2 * FEA:3 * FEA] = a["async_fea"]
    out[:, :, 3 * FEA:] = a["sync_fea"]
    for core in range(NCORES):
        Af, fb = meta[core]
        refined = _postprocess_core(results[core]["outT"], Af, fb)
        g = core % 4
        bs = slice(g * NB, (g + 1) * NB)
        col = slice(0, FEA) if core < 4 else slice(FEA, 2 * FEA)
        out[bs, :, col] = refined
    return out


def kernel(**inputs):
    from concourse import bass_utils

    nc = _get_program()
    a = {k: np.asarray(v) for k, v in inputs.items()}
    in_maps, meta = _make_in_maps(a)
    res = bass_utils.run_bass_kernel_spmd(nc, in_maps, core_ids=list(range(NCORES)))
    return _assemble(a, meta, res.results)
